# revision 24
# baseline (speedup 1.0000x reference)
"""Self-contained Trainium2 Bass kernel for nn_EncoderLayer_9216999817377. v4.

Encoder layer: QKV proj -> masked softmax attention -> add&LN ->
FFN (768->3072->768, no activation) -> add&LN.

Sharding: 8 cores = (batch b in 0..3) x (query half qh in 0..1). Pure SPMD
data parallelism, NO collectives: each core projects K/V for all 2048 keys
of its batch element itself.

v4 structural changes over v3 (sim 318us):
  - ACT does exp ONLY during attention (it is the pacer at ~8.3us/head);
    every other PSUM eviction is on DVE, SBUF elementwise on DVE/Pool.
    This also eliminates activation-table thrash on real HW (exp and sqrt
    live in different ACT table sets; all sqrts now batch at the tail).
  - LN1's rsqrt is eliminated algebraically: LN2(c*row)=LN2(row) and the
    FFN is linear, so only the mean-subtraction of LN1 matters. LN1 is a
    fused tensor_tensor_reduce (residual add + row sum) + subtract.
  - Batched DMAs (one per tensor region, need-ordered) instead of 45
    descriptors serialized on HWDGE at 625ns each.
  - Projections run inside the attention stream as fillers: prologue does
    K/Q for head 0; V and the rest pump between score tiles at a rate that
    keeps PE under the ACT pace. Per head, emission is [16 score/mask
    matmuls + 8 exp]; the 8 ctx matmuls and the transpose/normalize tail
    are deferred into the next head (t2/t4 hooks) so late V evictions and
    serial tails cannot stall the in-order PE queue ahead of the scores.
  - Leading DMA chain split fine (wk, wq, xtq quarters, first mask tiles)
    so head 0 starts ~7us in; exp0 is gated only by its own data.
  - Residual input, FFN residual, and output in bf16 (tolerance headroom),
    halving those DMAs and enabling DVE fast modes where eligible.
  - No mid-kernel pool closes (the v3 close barrier cost ~9us).
  - Final half-tail is pipelined across qt chains with evictions split
    ACT/DVE (ACT is idle once exp is done).

Pool-slot discipline: tiles in a pool share one untagged slot group of
`bufs` rotating slots, so every unit that allocates from a shared pool
(pPS1 especially) is emitted atomically: alloc -> use -> last read, with
no other allocation from that pool interleaved while the tile is live.
"""

from collections import deque
from contextlib import ExitStack

import numpy as np
import ml_dtypes

import concourse.bass as bass
import concourse.tile as tile
from concourse import mybir
from concourse.vector_clock import ScopedClock
from concourse.masks import make_identity

BF16 = mybir.dt.bfloat16
FP8 = mybir.dt.float8e4
F32 = mybir.dt.float32
AF = mybir.ActivationFunctionType
ALU = mybir.AluOpType
DRM = mybir.MatmulPerfMode.DoubleRow

B, S, D = 4, 2048, 768
H, DH = 12, 64
QC = 1024            # query rows per core
LN_EPS = 1e-5
NCORES = 8
MNEG = -240.0        # mask additive value (fp8e4 max finite is 240)


# ---------------------------------------------------------------------------
# Tile tail-drain patch: this container's walrus lowers CTRL (NoOp/Drain)
# instructions with a single sync-wait slot, but Tile's tail drain attaches
# one wait per live logical proc. Split the waits onto a chain of NOPs
# (1 wait each) emitted immediately before the drain on the SP stream.
def _patched_drain_and_barrier(self, tick_clock, wait_clock):
    carrier = self.nc.sync.nop(nofuse=True)
    wait_clock.add_sem_waits(carrier.ins, ScopedClock({None: tick_clock.global_clock}))
    si = carrier.ins.sync_info
    waits = list(si.on_wait) if si is not None else []
    carrier.ins.sync_info = mybir.SyncInfo(on_wait=waits[:1], on_update=[])
    for w in waits[1:]:
        n2 = self.nc.sync.nop(nofuse=True)
        n2.ins.sync_info = mybir.SyncInfo(on_wait=[w], on_update=[])
    self.nc.sync.drain()
    self.nc.all_engine_barrier()
    assert self.sems is not None
    popped = self.nc._tile_sem_poison_stack.pop()
    assert popped is self._sem_poison
    self.nc.clear_and_free_semaphores(list(self.sems.allocated().values()))
    self.nc.all_engine_barrier()


tile.TileContext._drain_and_barrier = _patched_drain_and_barrier


# This walrus also rejects >1 sync wait on regular engine instructions
# (setupSyncWait caps at one wait command per instruction). Tile's wait
# assignment packs up to two. Before lowering, split the extra wait onto a
# same-engine NoOp inserted immediately before the instruction -- the engine
# blocks at the NoOp instead, which is semantically identical.
_orig_lower_ordered = tile.TileContext._lower_ordered_insts


def _split_excess_waits(self, ordered):
    for bb_name, insts in ordered.items():
        out = []
        for inst in insts:
            si = getattr(inst, "sync_info", None)
            waits = list(si.on_wait) if si is not None else []
            if len(waits) > 1:
                for w in waits[:-1]:
                    nop = mybir.InstNoOp(
                        name=self.nc.get_next_instruction_name(), ins=[], outs=[])
                    nop.engine = inst.engine
                    nop.bass_nofuse = True
                    nop.sync_info = mybir.SyncInfo(on_wait=[w], on_update=[])
                    out.append(nop)
                inst.sync_info = mybir.SyncInfo(
                    on_wait=[waits[-1]], on_update=list(si.on_update))
            out.append(inst)
        ordered[bb_name] = out
    return _orig_lower_ordered(self, ordered)


tile.TileContext._lower_ordered_insts = _split_excess_waits


def build_nc():
    """Emit the per-core program. Identical on all 8 cores (SPMD)."""
    nc = bass.Bass("TRN2", target_bir_lowering=False, debug=False,
                   num_devices=NCORES)

    # ---- DRAM parameters (per-core shards, host-prepared) ----
    # fp8 plane layouts: row (c*128+p), col (i*F + f) maps to tile
    # [128, 2, F] with planes i = paired 128-contraction chunks.
    xtq_d = nc.declare_dram_parameter("xtq8", [D, S], FP8, isOutput=False)
    xq_d = nc.declare_dram_parameter("xq", [QC, D], BF16, isOutput=False)
    mb_d = nc.declare_dram_parameter("mask8", [1024, 2 * QC], FP8, isOutput=False)
    i2_d = nc.declare_dram_parameter("i2", [128, 512], FP8, isOutput=False)
    wq_d = nc.declare_dram_parameter("wq8", [D, D], FP8, isOutput=False)
    wk_d = nc.declare_dram_parameter("wk8", [D, D], FP8, isOutput=False)
    wv_d = nc.declare_dram_parameter("wv8", [D, D], FP8, isOutput=False)
    wfh_d = nc.declare_dram_parameter("wfh8", [D, D], FP8, isOutput=False)
    wfl_d = nc.declare_dram_parameter("wfl8", [D, D], FP8, isOutput=False)
    out_d = nc.declare_dram_parameter("out", [QC, D], BF16, isOutput=True)

    CD = D // 128          # 6 contraction chunks of 128 for D
    C2 = CD // 2           # 3 plane-pairs for D
    QT8 = QC // 128        # 8 query tiles of 128
    ST = S // 128          # 16 key tiles of 128
    SP = ST // 2           # 8 key-tile pairs
    VB = 128               # V head block: [64 V | 1 ones | 63 pad] (fp8
                           # dual ldweights wants 128-wide, x128 strides)

    with tile.TileContext(nc) as tc, ExitStack() as ctx:
        # ---------- kernel-long constants ----------
        const = ctx.enter_context(tc.tile_pool(name="const", bufs=1))
        ident = const.tile([128, 128], BF16, name="ident")
        make_identity(nc, ident)
        # half-zeroed doubled identities: i2[:, hf] has -240*delta rows
        # only in partition half hf, so the mask DR matmul is a full-128
        # weight load regardless of which kt parity it applies.
        i2 = const.tile([128, 2, 2, 128], FP8, name="i2")
        eps_t = const.tile([128, 1], F32, name="eps")
        nc.vector.memset(eps_t[:], LN_EPS)

        # ---------- kernel-long pools ----------
        pAT = ctx.enter_context(tc.tile_pool(name="pAT", bufs=1))
        pXQ = ctx.enter_context(tc.tile_pool(name="pXQ", bufs=1))
        AT_sb = [pAT.tile([128, D], BF16, name=f"at{i}") for i in range(QT8)]
        XQ_sb = pXQ.tile([128, 4, D], BF16, name="xq")
        # fused FFN weights: Ws = 32*(W1@W2), hi/lo fp8 error compensation
        pWF = ctx.enter_context(tc.tile_pool(name="pWF", bufs=1))
        WH_sb = pWF.tile([128, CD, D], FP8, name="wfh")
        WL_sb = pWF.tile([128, CD, D], FP8, name="wfl")
        WH_dr = [WH_sb[:, 2 * i:2 * i + 2] for i in range(C2)]
        WL_dr = [WL_sb[:, 2 * i:2 * i + 2] for i in range(C2)]
        pFT = ctx.enter_context(tc.tile_pool(name="pFT", bufs=1))
        FTH_dr = [pFT.tile([128, 2, QC], FP8, name=f"fth{i}") for i in range(C2)]
        FTL_dr = [pFT.tile([128, 2, QC], FP8, name=f"ftl{i}") for i in range(C2)]
        # persistent LN2 inputs + stats (normalized at the very end so every
        # sqrt batches away from exp's activation table)
        pU2 = ctx.enter_context(tc.tile_pool(name="pU2", bufs=1))
        U2_sb = [pU2.tile([128, D], BF16, name=f"u2_{i}") for i in range(QT8)]
        MV_sb = pU2.tile([128, QT8, 2], F32, name="mv")   # (mean, var) per qt
        RS_sb = pU2.tile([128, QT8], F32, name="rs2")     # 1/std per qt
        OT_sb = pU2.tile([128, 4, D], BF16, name="ot")
        pST = ctx.enter_context(tc.tile_pool(name="pST", bufs=2))
        pSC = ctx.enter_context(tc.tile_pool(name="pSC", bufs=6))
        pR2 = ctx.enter_context(tc.tile_pool(name="pR2", bufs=4))
        # PSUM: pPS (scores, 2x2 banks) + pPC (ctx, 2x1) + pPS1 (misc, 2x1)
        pPS = ctx.enter_context(tc.tile_pool(name="pPS", bufs=2, space="PSUM"))
        pPC = ctx.enter_context(tc.tile_pool(name="pPC", bufs=2, space="PSUM"))
        pPS1 = ctx.enter_context(tc.tile_pool(name="pPS1", bufs=2, space="PSUM"))

        # ---------- attention-long tensors ----------
        pKT = ctx.enter_context(tc.tile_pool(name="pKT", bufs=1))
        pQT = ctx.enter_context(tc.tile_pool(name="pQT", bufs=1))
        pV = ctx.enter_context(tc.tile_pool(name="pV", bufs=1))
        pMB = ctx.enter_context(tc.tile_pool(name="pMB", bufs=1))
        pXTQ = ctx.enter_context(tc.tile_pool(name="pXTQ", bufs=1))
        pWP = ctx.enter_context(tc.tile_pool(name="pWP", bufs=1))
        KT_sb = [pKT.tile([128, S], BF16, name=f"kt{i}") for i in range(CD)]
        QT_sb = [pQT.tile([128, QC], BF16, name=f"qt{i}") for i in range(CD)]
        V_dr = [pV.tile([128, 2, H * VB], FP8, name=f"v{i}") for i in range(SP)]
        # mask planes: tile t holds kt=2t in partitions 0..63 and kt=2t+1
        # in 64..127 (the half-zeroed i2 selects the active half)
        MB_sb = pMB.tile([128, SP, 2, QC], FP8, name="mb")
        MB_dr = [MB_sb[:, i] for i in range(SP)]
        XTQ_sb = pXTQ.tile([128, CD, S], FP8, name="xtq")
        WQ_sb = pWP.tile([128, CD, D], FP8, name="wq")
        WK_sb = pWP.tile([128, CD, D], FP8, name="wk")
        WV_sb = pWP.tile([128, CD, D], FP8, name="wv")

        pPT = ctx.enter_context(tc.tile_pool(name="pPT", bufs=10))
        pCX = ctx.enter_context(tc.tile_pool(name="pCX", bufs=2))
        pRS = ctx.enter_context(tc.tile_pool(name="pRS", bufs=4))

        def wplane(dram):
            return dram[:].rearrange("(rb p) f -> p rb f", p=128)

        xtq_r = xtq_d[:].rearrange("(rb p) f -> p rb f", p=128)
        mb_r = mb_d[:].rearrange("(t p) (i f) -> p t i f", p=128, i=2)

        # ---------- DMAs: batched, in order of first use; the leading
        # chain is split fine so head 0 can start ~7us in ----------
        nc.sync.dma_start(out=WK_sb[:], in_=wplane(wk_d))
        nc.sync.dma_start(out=WQ_sb[:], in_=wplane(wq_d))
        nc.sync.dma_start(out=XTQ_sb[:, :, 0:512], in_=xtq_r[:, :, 0:512])
        nc.sync.dma_start(out=i2[:], in_=i2_d[:].rearrange(
            "p (hf i f) -> p hf i f", hf=2, i=2))
        nc.sync.dma_start(out=MB_sb[:, 0:1], in_=mb_r[:, 0:1])
        nc.sync.dma_start(out=XTQ_sb[:, :, 512:QC], in_=xtq_r[:, :, 512:QC])
        nc.sync.dma_start(out=MB_sb[:, 1:2], in_=mb_r[:, 1:2])
        nc.sync.dma_start(out=WV_sb[:], in_=wplane(wv_d))
        nc.sync.dma_start(out=MB_sb[:, 2:4], in_=mb_r[:, 2:4])
        nc.sync.dma_start(out=XTQ_sb[:, :, QC:QC + 512],
                          in_=xtq_r[:, :, QC:QC + 512])
        nc.sync.dma_start(out=MB_sb[:, 4:6], in_=mb_r[:, 4:6])
        nc.sync.dma_start(out=XTQ_sb[:, :, QC + 512:S],
                          in_=xtq_r[:, :, QC + 512:S])
        nc.sync.dma_start(out=MB_sb[:, 6:8], in_=mb_r[:, 6:8])
        nc.sync.dma_start(out=XQ_sb[:], in_=xq_d[0:512].rearrange(
            "(q p) d -> p q d", p=128))
        nc.sync.dma_start(out=WH_sb[:], in_=wplane(wfh_d))
        nc.sync.dma_start(out=WL_sb[:], in_=wplane(wfl_d))

        # V pad memsets (Pool, runs from t=0; only the 63 pad cols + ones)
        for sp in range(SP):
            vre = V_dr[sp].rearrange("p i (h c) -> p i h c", c=VB)
            nc.gpsimd.memset(vre[:, :, :, DH + 1:], 0.0)
            nc.gpsimd.memset(vre[:, :, :, DH:DH + 1], 1.0)

        # ---------- projection work units (each atomic in pPS1) ----------
        def k_unit(dt, kc, act):
            """KT[dt][:, kc*512:+512] from a 3-matmul DR chain."""
            ps = pPS1.tile([128, 512], F32, name="ps1")
            for c in range(C2):
                nc.tensor.matmul(
                    ps[:],
                    lhsT=WK_sb[:, 2 * c:2 * c + 2, dt * 128:(dt + 1) * 128],
                    rhs=XTQ_sb[:, 2 * c:2 * c + 2, kc * 512:(kc + 1) * 512],
                    start=(c == 0), stop=(c == C2 - 1), perf_mode=DRM)
            dst = KT_sb[dt][:, kc * 512:(kc + 1) * 512]
            if act:
                nc.scalar.copy(dst, ps[:])
            else:
                nc.vector.tensor_copy(dst, ps[:])

        def q_unit(dt, qc, act):
            ps = pPS1.tile([128, 512], F32, name="ps1")
            for c in range(C2):
                nc.tensor.matmul(
                    ps[:],
                    lhsT=WQ_sb[:, 2 * c:2 * c + 2, dt * 128:(dt + 1) * 128],
                    rhs=XTQ_sb[:, 2 * c:2 * c + 2, qc * 512:(qc + 1) * 512],
                    start=(c == 0), stop=(c == C2 - 1), perf_mode=DRM)
            dst = QT_sb[dt][:, qc * 512:(qc + 1) * 512]
            if act:
                nc.scalar.copy(dst, ps[:])
            else:
                nc.vector.tensor_copy(dst, ps[:])

        def v_unit(st, act):
            """V rows for key tile st (both 384-col halves)."""
            for j in range(2):
                ps = pPS1.tile([128, 512], F32, name="ps1")[:, 0:384]
                for c in range(C2):
                    nc.tensor.matmul(
                        ps[:],
                        lhsT=XTQ_sb[:, 2 * c:2 * c + 2, st * 128:(st + 1) * 128],
                        rhs=WV_sb[:, 2 * c:2 * c + 2, j * 384:(j + 1) * 384],
                        start=(c == 0), stop=(c == C2 - 1), perf_mode=DRM)
                dst = V_dr[st // 2].rearrange(
                    "p i (h c) -> p i h c", c=VB)[
                    :, st % 2, j * 6:(j + 1) * 6, 0:DH]
                src = ps.rearrange("p (h c) -> p h c", c=DH)
                if act:
                    nc.scalar.copy(dst, src)
                else:
                    nc.vector.tensor_copy(dst, src)

        # ---------- attention ----------
        cxr = {}

        def attention(h, qc, pump, at2, at4):
            """One head: 16 mask/score matmuls + 8 exp. The ctx matmuls and
            the transpose/normalize tail are closures emitted during the
            NEXT head (at2/at4 hooks) so V-eviction lag and serial tails
            cannot stall the in-order PE queue ahead of these scores."""
            dtile, doff = h // 2, (h % 2) * DH
            qsl = slice(qc * 512, (qc + 1) * 512)
            pts = []
            for t in range(SP):
                ps = pPS.tile([128, 2, 512], F32, name="ps")
                for j in range(2):
                    kt = 2 * t + j
                    nc.tensor.matmul(
                        ps[:, j, :],
                        lhsT=i2[:, j, :, :],
                        rhs=MB_dr[t][:, :, qsl],
                        start=True, stop=False, perf_mode=DRM)
                    nc.tensor.matmul(
                        ps[:, j, :],
                        lhsT=KT_sb[dtile][doff:doff + DH,
                                          kt * 128:(kt + 1) * 128],
                        rhs=QT_sb[dtile][doff:doff + DH, qsl],
                        start=False, stop=True)
                pt = pPT.tile([128, 2, 512], FP8, name="pt")
                nc.scalar.activation(pt[:], ps[:], AF.Exp, scale=1.0 / 8.0)
                pts.append(pt)
                if t == 2 and at2 is not None:
                    at2()
                if t == 4 and at4 is not None:
                    at4()
                pump(t)

            def ctxf(h=h, pts=pts):
                pc = pPC.tile([128, 512], F32, name="pc")
                for t in range(SP):
                    nc.tensor.matmul(
                        pc[:],
                        lhsT=V_dr[t][:, :, h * VB:(h + 1) * VB],
                        rhs=pts[t][:],
                        start=(t == 0), stop=(t == SP - 1), perf_mode=DRM)
                # ctx^T: rows 0..63 ctx, row 64 softmax denominators
                cx = pCX.tile([128, 512], BF16, name="cx")
                nc.vector.tensor_copy(cx[:], pc[:])
                cxr[h] = cx

            def tail(h=h, qc=qc):
                cx = cxr.pop(h)
                tp = pPS1.tile([128, 512], BF16, name="ps1")
                for qtl in range(4):
                    nc.tensor.transpose(
                        tp[:, qtl * 128:(qtl + 1) * 128],
                        cx[:, qtl * 128:(qtl + 1) * 128], ident[:])
                rs = pRS.tile([128, 4], F32, name="rs")
                for qtl in range(4):
                    qt = qc * 4 + qtl
                    nc.vector.reciprocal(rs[:, qtl:qtl + 1],
                                         tp[:, qtl * 128 + DH:qtl * 128 + DH + 1])
                    nc.vector.tensor_scalar_mul(
                        AT_sb[qt][:, h * DH:(h + 1) * DH],
                        tp[:, qtl * 128:qtl * 128 + DH], rs[:, qtl:qtl + 1])
            return ctxf, tail

        # ---------- LN1 + FFN + LN2-input chunks per query tile ----------
        def c1a_unit(qt):
            """u = AT+XQ; U2[qt] = bf16(u-mean) [doubles as the FFN
            residual]. LN1 needs no variance: the 1/std scale cancels
            through the linear FFN into scale-invariant LN2."""
            sm = pST.tile([128, 2], F32, name="sm")
            nc.vector.tensor_tensor(AT_sb[qt][:], AT_sb[qt][:],
                                    XQ_sb[:, qt % 4], ALU.add)
            nc.vector.tensor_reduce(sm[:, 0:1], AT_sb[qt][:],
                                    mybir.AxisListType.X, ALU.add)
            nc.gpsimd.tensor_scalar_mul(sm[:, 1:2], sm[:, 0:1], 1.0 / D)
            nc.vector.tensor_scalar(U2_sb[qt][:], AT_sb[qt][:], sm[:, 1:2],
                                    None, ALU.subtract)

        def h8_unit(qt, pool_lb):
            h8 = pSC.tile([128, D], FP8, name="h8")
            nc.gpsimd.tensor_copy(h8[:], U2_sb[qt][:])
            lb = pSC.tile([128, D], BF16, name="lb")
            if pool_lb:
                nc.gpsimd.tensor_tensor(lb[:], U2_sb[qt][:], h8[:],
                                        ALU.subtract)
            else:
                nc.vector.tensor_tensor(lb[:], U2_sb[qt][:], h8[:],
                                        ALU.subtract)
            return lb

        def c1_unit(qt):
            c1a_unit(qt)
            return h8_unit(qt, pool_lb=True)

        def c2_one(qt, src_t, dr, cp, act):
            """Transpose d-chunk pair (2cp, 2cp+1) of src into both planes
            of dr[cp]: 2 transposes into one PSUM tile, 1 paired evict."""
            tp = pPS1.tile([128, 512], BF16, name="ps1")
            for k in range(2):
                c = 2 * cp + k
                nc.tensor.transpose(
                    tp[:, k * 128:(k + 1) * 128],
                    src_t[:, c * 128:(c + 1) * 128], ident[:])
            psrc = tp[:, 0:256].rearrange("p (i f) -> p i f", i=2)
            pdst = dr[cp][:, :, qt * 128:(qt + 1) * 128]
            if act:
                nc.scalar.copy(pdst, psrc)
            else:
                nc.vector.tensor_copy(pdst, psrc)

        def c2_unit(qt, lb, cp, act):
            c2_one(qt, U2_sb[qt], FTH_dr, cp, act)
            c2_one(qt, lb, FTL_dr, cp, act)

        def c3_units(qt, act=False):
            """FFN j-halves as 2 atomic PE units + a stats closure."""
            r2 = pR2.tile([128, D], BF16, name="r2")

            def mmj(j, qt=qt):
                def go():
                    ps = pPS1.tile([128, 512], F32, name="ps1")[:, 0:384]
                    kk = 0
                    for (A, W) in ((FTH_dr, WH_dr), (FTH_dr, WL_dr),
                                   (FTL_dr, WH_dr)):
                        for c in range(C2):
                            nc.tensor.matmul(
                                ps[:],
                                lhsT=A[c][:, :, qt * 128:(qt + 1) * 128],
                                rhs=W[c][:, :, j * 384:(j + 1) * 384],
                                start=(kk == 0), stop=(kk == 3 * C2 - 1),
                                perf_mode=DRM)
                            kk += 1
                    if act:
                        nc.scalar.activation(
                            r2[:, j * 384:(j + 1) * 384], ps[:], AF.Copy,
                            scale=1.0 / 32.0)
                    else:
                        nc.vector.tensor_scalar_mul(
                            r2[:, j * 384:(j + 1) * 384], ps[:], 1.0 / 32.0)
                return go

            def fin(qt=qt):
                nc.vector.tensor_tensor(U2_sb[qt][:], U2_sb[qt][:], r2[:],
                                        ALU.add)
                st6 = pST.tile([128, 2, 6], F32, name="st6")
                nc.vector.bn_stats(st6[:, 0, :], U2_sb[qt][:, 0:D // 2])
                nc.vector.bn_stats(st6[:, 1, :], U2_sb[qt][:, D // 2:])
                nc.vector.bn_aggr(MV_sb[:, qt], st6[:])
            return [mmj(0), mmj(1), fin]

        def ln2_apply(qt):
            """(u2 - mean) * rstd -> OT (bf16). qt0-3 run on ACT right
            after the last exp (fills its bubble); qt4-7 on DVE so the
            last chains don't serialize behind ACT's eviction queue."""
            if qt < 4:
                nm = pRS.tile([128, 1], F32, name="nm")
                nc.vector.tensor_scalar(nm[:], MV_sb[:, qt, 0:1],
                                        RS_sb[:, qt:qt + 1], -1.0,
                                        ALU.mult, ALU.mult)
                nc.scalar.activation(OT_sb[:, qt % 4], U2_sb[qt][:],
                                     AF.Identity,
                                     scale=RS_sb[:, qt:qt + 1], bias=nm[:])
            else:
                nc.vector.tensor_scalar(OT_sb[:, qt % 4], U2_sb[qt][:],
                                        MV_sb[:, qt, 0:1],
                                        RS_sb[:, qt:qt + 1],
                                        ALU.subtract, ALU.mult)

        # ---------- emission schedule ----------
        # Prologue: K dt0 cols 0:1024 + Q dt0 qsl0 (ACT evicts; ACT idle).
        k_unit(0, 0, act=True)
        q_unit(0, 0, act=True)
        k_unit(0, 1, act=False)

        # h0 fillers: V st0-13 at 2/t, K dt0 kc2-3 squeezed on t2-3;
        # V st14-15 land on h1 t0-1, just before ctx(h0) fires at h1 t2.
        h0_sched = {
            0: [lambda: v_unit(0, False), lambda: v_unit(1, False)],
            1: [lambda: v_unit(2, False), lambda: v_unit(3, False)],
            2: [lambda: v_unit(4, False), lambda: k_unit(0, 2, False)],
            3: [lambda: v_unit(5, False), lambda: k_unit(0, 3, False)],
            4: [lambda: v_unit(6, False), lambda: v_unit(7, False)],
            5: [lambda: v_unit(8, False), lambda: v_unit(9, False)],
            6: [lambda: v_unit(10, False), lambda: v_unit(11, False)],
            7: [lambda: v_unit(12, False), lambda: v_unit(13, False)],
        }
        h1_sched = {
            0: [lambda: v_unit(14, False)],
            1: [lambda: v_unit(15, False)],
        }

        fillers = deque()
        fillers.append(lambda: q_unit(0, 1, False))
        for dt in range(1, CD):
            fillers.append(lambda dt=dt: q_unit(dt, 0, False))
            fillers.append(lambda dt=dt: q_unit(dt, 1, False))
            for kc in range(4):
                fillers.append(lambda dt=dt, kc=kc: k_unit(dt, kc, False))

        def pump(t):
            if fillers:
                fillers.popleft()()

        def pump_slow(t):
            if t % 2 and fillers:
                fillers.popleft()()

        def pump_h0(t):
            for u in h0_sched.get(t, ()):
                u()

        def pump_h1(t):
            for u in h1_sched.get(t, ()):
                u()
            pump(t)

        carry = {}

        def queue_halftail(qh):
            for qtl in range(4):
                qt = qh * 4 + qtl

                def c1c(qt=qt):
                    carry[qt] = c1_unit(qt)

                def c3all(qt=qt):
                    carry.pop(qt)
                    for u in reversed(c3_units(qt)):
                        fillers.appendleft(u)

                fillers.append(c1c)
                for cp in range(3):
                    fillers.append(
                        lambda qt=qt, cp=cp: c2_unit(qt, carry[qt], cp,
                                                     act=False))
                fillers.append(c3all)

        p_ctx, p_tail = None, None
        for qc in (0, 1):
            for h in range(H):
                if qc == 0 and h == 0:
                    pmp = pump_h0
                elif qc == 0 and h == 1:
                    pmp = pump_h1
                elif qc == 0:
                    pmp = pump_slow
                else:
                    pmp = pump
                p_ctx, p_tail = attention(h, qc, pmp, p_ctx, p_tail)
                if qc == 1 and h == 0:
                    queue_halftail(0)
                if qc == 1 and h == 6:
                    # refill the residual buffer with the second query half
                    nc.sync.dma_start(
                        out=XQ_sb[:], in_=xq_d[512:1024].rearrange(
                            "(q p) d -> p q d", p=128))

        # last ctx + tail; then finish qt0-3 (their stats are done) while
        # the qt4-7 chains run, all sqrts batched in one ACT table context.
        p_ctx()
        p_tail()
        while fillers:
            fillers.popleft()()
        sd03 = pRS.tile([128, 4], F32, name="sd03")
        nc.scalar.activation(sd03[:], MV_sb[:, 0:4, 1], AF.Sqrt, bias=eps_t[:])
        nc.vector.reciprocal(RS_sb[:, 0:4], sd03[:])
        for qt in range(4):
            ln2_apply(qt)
        nc.sync.dma_start(
            out=out_d[0:512, :].rearrange("(q p) d -> p q d", p=128),
            in_=OT_sb[:])
        # qt4-7 chains, stage round-robin: c1a -> hb-transposes (run
        # while h8/lb compute) -> lb-transposes -> FFN -> stats -> LN2
        QF = (4, 5, 6, 7)
        for qt in QF:
            c1a_unit(qt)
        lbs = {}
        lbs[4] = h8_unit(4, pool_lb=True)
        for cp in range(3):
            c2_one(4, U2_sb[4], FTH_dr, cp, act=True)
        lbs[5] = h8_unit(5, pool_lb=False)
        for cp in range(3):
            c2_one(5, U2_sb[5], FTH_dr, cp, act=True)
        lbs[6] = h8_unit(6, pool_lb=True)
        for cp in range(3):
            c2_one(4, lbs[4], FTL_dr, cp, act=True)
        lbs[7] = h8_unit(7, pool_lb=False)
        for cp in range(3):
            c2_one(6, U2_sb[6], FTH_dr, cp, act=True)
        c3u = {qt: c3_units(qt, act=True) for qt in QF}
        c3u[4][0]()
        for cp in range(3):
            c2_one(5, lbs[5], FTL_dr, cp, act=True)
        c3u[4][1]()
        for cp in range(3):
            c2_one(7, U2_sb[7], FTH_dr, cp, act=True)
        c3u[5][0]()
        for cp in range(3):
            c2_one(6, lbs[6], FTL_dr, cp, act=True)
        c3u[5][1]()
        c3u[4][2]()
        for cp in range(3):
            c2_one(7, lbs[7], FTL_dr, cp, act=True)
        c3u[6][0]()
        c3u[6][1]()
        c3u[5][2]()

        def fin2(qt):
            sd = pRS.tile([128, 1], F32, name="sd")
            nc.scalar.activation(sd[:], MV_sb[:, qt, 1:2], AF.Sqrt,
                                 bias=eps_t[:])
            nc.vector.reciprocal(RS_sb[:, qt:qt + 1], sd[:])
            ln2_apply(qt)

        fin2(4)
        c3u[7][0]()
        c3u[7][1]()
        c3u[6][2]()
        fin2(5)
        c3u[7][2]()
        fin2(6)
        fin2(7)
        nc.sync.dma_start(
            out=out_d[512:1024, :].rearrange("(q p) d -> p q d", p=128),
            in_=OT_sb[:])

    return nc


_built = {}


def _get_nc():
    if "nc" not in _built:
        _built["nc"] = build_nc()
    return _built["nc"]


def _make_in_maps(inputs):
    f8 = ml_dtypes.float8_e4m3
    x = np.asarray(inputs["inputs"], np.float32)
    mask = np.asarray(inputs["attn_mask"], bool)
    wq = np.asarray(inputs["Wq"], np.float32).astype(f8)
    wk = np.asarray(inputs["Wk"], np.float32).astype(f8)
    wv = np.asarray(inputs["Wv"], np.float32).astype(f8)
    # fused linear FFN: Ws = 32*(W1@W2), hi/lo fp8 split for compensation
    ws = 32.0 * (np.asarray(inputs["W1"], np.float32)
                 @ np.asarray(inputs["W2"], np.float32))
    wh = ws.astype(f8)
    wl = (ws - wh.astype(np.float32)).astype(f8)
    wfh, wfl = wh, wl

    i2 = np.zeros((128, 2, 2, 128), np.float32)
    for p in range(128):
        for i in range(2):
            i2[p, p // 64, i, i * 64 + p % 64] = MNEG
    i2 = np.ascontiguousarray(i2.reshape(128, 512)).astype(f8)

    # The device program folds no affine terms: this module's initialization
    # has them all zero / one -- assert the cases we don't emit.
    for name in ("bq", "bk", "bv", "b1", "b2", "beta1", "beta2"):
        assert not np.asarray(inputs[name]).any(), f"{name} nonzero unsupported"
    for name in ("g1", "g2"):
        assert np.allclose(np.asarray(inputs[name]), 1.0), f"{name} != 1 unsupported"

    in_maps = []
    for core in range(NCORES):
        b, qh = core // 2, core % 2
        q0 = qh * QC
        # Per-core sequence permutation: own queries first (cols 0..1023) so
        # the SPMD program can slice Q at a fixed offset. K/V/mask all use
        # the same permuted key order; softmax is key-order invariant.
        perm = np.r_[np.arange(q0, q0 + QC),
                     np.arange(0, q0),
                     np.arange(q0 + QC, S)]
        xtq8 = np.ascontiguousarray(x[b].T[:, perm]).astype(f8)
        xq = np.ascontiguousarray(x[b, q0:q0 + QC]).astype(ml_dtypes.bfloat16)
        # mask8[t*128 + j*64 + r, i*1024 + q] = mkq[(2t+j)*128 + i*64+r, q]
        mkq = mask[b][q0:q0 + QC][:, perm].T.astype(np.float32)  # [k(perm), q]
        m5 = mkq.reshape(8, 2, 2, 64, QC).transpose(0, 1, 3, 2, 4)
        mask8 = np.ascontiguousarray(m5.reshape(1024, 2 * QC)).astype(f8)
        in_maps.append(dict(xtq8=xtq8, xq=xq, mask8=mask8, i2=i2,
                            wq8=wq, wk8=wk, wv8=wv, wfh8=wfh, wfl8=wfl))
    return in_maps


def _run(in_maps):
    from concourse.bass_utils import run_bass_kernel_spmd
    nc = _get_nc()
    return run_bass_kernel_spmd(nc, in_maps, list(range(NCORES)))


def _assemble(res):
    out = np.empty((B, S, D), np.float32)
    for core in range(NCORES):
        b, qh = core // 2, core % 2
        out[b, qh * QC:(qh + 1) * QC] = res.results[core]["out"].astype(
            np.float32)
    return out


def kernel(**inputs) -> np.ndarray:
    return _assemble(_run(_make_in_maps(inputs)))


# revision 25
# speedup vs baseline: 1.2527x; 1.2527x over previous
"""Self-contained Trainium2 Bass kernel for nn_EncoderLayer_9216999817377. v4.

Encoder layer: QKV proj -> masked softmax attention -> add&LN ->
FFN (768->3072->768, no activation) -> add&LN.

Sharding: 8 cores = (batch b in 0..3) x (query half qh in 0..1). Pure SPMD
data parallelism, NO collectives: each core projects K/V for all 2048 keys
of its batch element itself.

v4 structural changes over v3 (sim 318us):
  - ACT does exp ONLY during attention (it is the pacer at ~8.3us/head);
    every other PSUM eviction is on DVE, SBUF elementwise on DVE/Pool.
    This also eliminates activation-table thrash on real HW (exp and sqrt
    live in different ACT table sets; all sqrts now batch at the tail).
  - LN1's rsqrt is eliminated algebraically: LN2(c*row)=LN2(row) and the
    FFN is linear, so only the mean-subtraction of LN1 matters. LN1 is a
    fused tensor_tensor_reduce (residual add + row sum) + subtract.
  - Batched DMAs (one per tensor region, need-ordered) instead of 45
    descriptors serialized on HWDGE at 625ns each.
  - Projections run inside the attention stream as fillers: prologue does
    K/Q for head 0; V and the rest pump between score tiles at a rate that
    keeps PE under the ACT pace. Per head, emission is [16 score/mask
    matmuls + 8 exp]; the 8 ctx matmuls and the transpose/normalize tail
    are deferred into the next head (t2/t4 hooks) so late V evictions and
    serial tails cannot stall the in-order PE queue ahead of the scores.
  - Leading DMA chain split fine (wk, wq, xtq quarters, first mask tiles)
    so head 0 starts ~7us in; exp0 is gated only by its own data.
  - Residual input, FFN residual, and output in bf16 (tolerance headroom),
    halving those DMAs and enabling DVE fast modes where eligible.
  - No mid-kernel pool closes (the v3 close barrier cost ~9us).
  - Final half-tail is pipelined across qt chains with evictions split
    ACT/DVE (ACT is idle once exp is done).

Pool-slot discipline: tiles in a pool share one untagged slot group of
`bufs` rotating slots, so every unit that allocates from a shared pool
(pPS1 especially) is emitted atomically: alloc -> use -> last read, with
no other allocation from that pool interleaved while the tile is live.
"""

from collections import deque
from contextlib import ExitStack

import numpy as np
import ml_dtypes

import concourse.bass as bass
import concourse.tile as tile
from concourse import mybir
from concourse.vector_clock import ScopedClock
from concourse.masks import make_identity

BF16 = mybir.dt.bfloat16
FP8 = mybir.dt.float8e4
F32 = mybir.dt.float32
AF = mybir.ActivationFunctionType
ALU = mybir.AluOpType
DRM = mybir.MatmulPerfMode.DoubleRow

B, S, D = 4, 2048, 768
H, DH = 12, 64
QC = 1024            # query rows per core
LN_EPS = 1e-5
NCORES = 8
MNEG = -240.0        # mask additive value (fp8e4 max finite is 240)


# ---------------------------------------------------------------------------
# Tile tail-drain patch: this container's walrus lowers CTRL (NoOp/Drain)
# instructions with a single sync-wait slot, but Tile's tail drain attaches
# one wait per live logical proc. Split the waits onto a chain of NOPs
# (1 wait each) emitted immediately before the drain on the SP stream.
def _patched_drain_and_barrier(self, tick_clock, wait_clock):
    carrier = self.nc.sync.nop(nofuse=True)
    wait_clock.add_sem_waits(carrier.ins, ScopedClock({None: tick_clock.global_clock}))
    si = carrier.ins.sync_info
    waits = list(si.on_wait) if si is not None else []
    carrier.ins.sync_info = mybir.SyncInfo(on_wait=waits[:1], on_update=[])
    for w in waits[1:]:
        n2 = self.nc.sync.nop(nofuse=True)
        n2.ins.sync_info = mybir.SyncInfo(on_wait=[w], on_update=[])
    self.nc.sync.drain()
    self.nc.all_engine_barrier()
    assert self.sems is not None
    popped = self.nc._tile_sem_poison_stack.pop()
    assert popped is self._sem_poison
    self.nc.clear_and_free_semaphores(list(self.sems.allocated().values()))
    self.nc.all_engine_barrier()


tile.TileContext._drain_and_barrier = _patched_drain_and_barrier


# This walrus also rejects >1 sync wait on regular engine instructions
# (setupSyncWait caps at one wait command per instruction). Tile's wait
# assignment packs up to two. Before lowering, split the extra wait onto a
# same-engine NoOp inserted immediately before the instruction -- the engine
# blocks at the NoOp instead, which is semantically identical.
_orig_lower_ordered = tile.TileContext._lower_ordered_insts


def _split_excess_waits(self, ordered):
    for bb_name, insts in ordered.items():
        out = []
        for inst in insts:
            si = getattr(inst, "sync_info", None)
            waits = list(si.on_wait) if si is not None else []
            if len(waits) > 1:
                for w in waits[:-1]:
                    nop = mybir.InstNoOp(
                        name=self.nc.get_next_instruction_name(), ins=[], outs=[])
                    nop.engine = inst.engine
                    nop.bass_nofuse = True
                    nop.sync_info = mybir.SyncInfo(on_wait=[w], on_update=[])
                    out.append(nop)
                inst.sync_info = mybir.SyncInfo(
                    on_wait=[waits[-1]], on_update=list(si.on_update))
            out.append(inst)
        ordered[bb_name] = out
    return _orig_lower_ordered(self, ordered)


tile.TileContext._lower_ordered_insts = _split_excess_waits


def build_nc():
    """Emit the per-core program. Identical on all 8 cores (SPMD)."""
    nc = bass.Bass("TRN2", target_bir_lowering=False, debug=False,
                   num_devices=NCORES)

    # ---- DRAM parameters (per-core shards, host-prepared) ----
    # fp8 plane layouts: row (c*128+p), col (i*F + f) maps to tile
    # [128, 2, F] with planes i = paired 128-contraction chunks.
    xtq_d = nc.declare_dram_parameter("xtq8", [D, S], FP8, isOutput=False)
    xq_d = nc.declare_dram_parameter("xq", [QC, D], BF16, isOutput=False)
    mb_d = nc.declare_dram_parameter("mask8", [1024, 2 * QC], FP8, isOutput=False)
    i2_d = nc.declare_dram_parameter("i2", [128, 512], FP8, isOutput=False)
    wq_d = nc.declare_dram_parameter("wq8", [D, D], FP8, isOutput=False)
    wk_d = nc.declare_dram_parameter("wk8", [D, D], FP8, isOutput=False)
    wv_d = nc.declare_dram_parameter("wv8", [D, D], FP8, isOutput=False)
    wfh_d = nc.declare_dram_parameter("wfh8", [D, D], FP8, isOutput=False)
    wfl_d = nc.declare_dram_parameter("wfl8", [D, D], FP8, isOutput=False)
    out_d = nc.declare_dram_parameter("out", [QC, D], BF16, isOutput=True)

    CD = D // 128          # 6 contraction chunks of 128 for D
    C2 = CD // 2           # 3 plane-pairs for D
    QT8 = QC // 128        # 8 query tiles of 128
    ST = S // 128          # 16 key tiles of 128
    SP = ST // 2           # 8 key-tile pairs
    VB = 128               # V head block: [64 V | 1 ones | 63 pad] (fp8
                           # dual ldweights wants 128-wide, x128 strides)

    with tile.TileContext(nc) as tc, ExitStack() as ctx:
        # ---------- kernel-long constants ----------
        const = ctx.enter_context(tc.tile_pool(name="const", bufs=1))
        ident = const.tile([128, 128], BF16, name="ident")
        make_identity(nc, ident)
        # half-zeroed doubled identities: i2[:, hf] has -240*delta rows
        # only in partition half hf, so the mask DR matmul is a full-128
        # weight load regardless of which kt parity it applies.
        i2 = const.tile([128, 2, 2, 128], FP8, name="i2")
        eps_t = const.tile([128, 1], F32, name="eps")
        nc.vector.memset(eps_t[:], LN_EPS)

        # ---------- kernel-long pools ----------
        pAT = ctx.enter_context(tc.tile_pool(name="pAT", bufs=1))
        pXQ = ctx.enter_context(tc.tile_pool(name="pXQ", bufs=1))
        AT_sb = [pAT.tile([128, D], BF16, name=f"at{i}") for i in range(QT8)]
        XQ_sb = pXQ.tile([128, 4, D], BF16, name="xq")
        # fused FFN weights: Ws = 32*(W1@W2), hi/lo fp8 error compensation
        pWF = ctx.enter_context(tc.tile_pool(name="pWF", bufs=1))
        WH_sb = pWF.tile([128, CD, D], FP8, name="wfh")
        WL_sb = pWF.tile([128, CD, D], FP8, name="wfl")
        WH_dr = [WH_sb[:, 2 * i:2 * i + 2] for i in range(C2)]
        WL_dr = [WL_sb[:, 2 * i:2 * i + 2] for i in range(C2)]
        pFT = ctx.enter_context(tc.tile_pool(name="pFT", bufs=1))
        FTH_dr = [pFT.tile([128, 2, QC], FP8, name=f"fth{i}") for i in range(C2)]
        FTL_dr = [pFT.tile([128, 2, QC], FP8, name=f"ftl{i}") for i in range(C2)]
        # persistent LN2 inputs + stats (normalized at the very end so every
        # sqrt batches away from exp's activation table)
        pU2 = ctx.enter_context(tc.tile_pool(name="pU2", bufs=1))
        U2_sb = [pU2.tile([128, D], BF16, name=f"u2_{i}") for i in range(QT8)]
        MV_sb = pU2.tile([128, QT8, 2], F32, name="mv")   # (mean, var) per qt
        RS_sb = pU2.tile([128, QT8], F32, name="rs2")     # 1/std per qt
        OT_sb = pU2.tile([128, 4, D], BF16, name="ot")
        pST = ctx.enter_context(tc.tile_pool(name="pST", bufs=2))
        pSC = ctx.enter_context(tc.tile_pool(name="pSC", bufs=4))
        pR2 = ctx.enter_context(tc.tile_pool(name="pR2", bufs=4))
        # PSUM: pPS (scores, 2x2 banks) + pPC (ctx, 2x1) + pPS1 (misc, 2x1)
        pPS = ctx.enter_context(tc.tile_pool(name="pPS", bufs=2, space="PSUM"))
        pPC = ctx.enter_context(tc.tile_pool(name="pPC", bufs=2, space="PSUM"))
        pPS1 = ctx.enter_context(tc.tile_pool(name="pPS1", bufs=2, space="PSUM"))

        # ---------- attention-long tensors ----------
        pKT = ctx.enter_context(tc.tile_pool(name="pKT", bufs=1))
        pQT = ctx.enter_context(tc.tile_pool(name="pQT", bufs=1))
        pV = ctx.enter_context(tc.tile_pool(name="pV", bufs=1))
        pMB = ctx.enter_context(tc.tile_pool(name="pMB", bufs=1))
        pXTQ = ctx.enter_context(tc.tile_pool(name="pXTQ", bufs=1))
        pWP = ctx.enter_context(tc.tile_pool(name="pWP", bufs=1))
        KT_sb = [pKT.tile([128, S], BF16, name=f"kt{i}") for i in range(CD)]
        QT_sb = [pQT.tile([128, QC], BF16, name=f"qt{i}") for i in range(CD)]
        V_dr = [pV.tile([128, 2, H * VB], FP8, name=f"v{i}") for i in range(SP)]
        # mask planes: tile t holds kt=2t in partitions 0..63 and kt=2t+1
        # in 64..127 (the half-zeroed i2 selects the active half)
        MB_sb = pMB.tile([128, SP, 2, QC], FP8, name="mb")
        MB_dr = [MB_sb[:, i] for i in range(SP)]
        XTQ_sb = pXTQ.tile([128, CD, S], FP8, name="xtq")
        WQ_sb = pWP.tile([128, CD, D], FP8, name="wq")
        WK_sb = pWP.tile([128, CD, D], FP8, name="wk")
        WV_sb = pWP.tile([128, CD, D], FP8, name="wv")

        pPT = ctx.enter_context(tc.tile_pool(name="pPT", bufs=10))
        pCX = ctx.enter_context(tc.tile_pool(name="pCX", bufs=2))
        pRS = ctx.enter_context(tc.tile_pool(name="pRS", bufs=4))

        def wplane(dram):
            return dram[:].rearrange("(rb p) f -> p rb f", p=128)

        xtq_r = xtq_d[:].rearrange("(rb p) f -> p rb f", p=128)
        mb_r = mb_d[:].rearrange("(t p) (i f) -> p t i f", p=128, i=2)

        # ---------- DMAs: batched, in order of first use; the leading
        # chain is split fine so head 0 can start ~7us in ----------
        nc.sync.dma_start(out=WK_sb[:], in_=wplane(wk_d))
        nc.sync.dma_start(out=WQ_sb[:], in_=wplane(wq_d))
        nc.sync.dma_start(out=XTQ_sb[:, :, 0:512], in_=xtq_r[:, :, 0:512])
        nc.sync.dma_start(out=i2[:], in_=i2_d[:].rearrange(
            "p (hf i f) -> p hf i f", hf=2, i=2))
        nc.sync.dma_start(out=MB_sb[:, 0:1], in_=mb_r[:, 0:1])
        nc.sync.dma_start(out=XTQ_sb[:, :, 512:QC], in_=xtq_r[:, :, 512:QC])
        nc.sync.dma_start(out=MB_sb[:, 1:2], in_=mb_r[:, 1:2])
        nc.sync.dma_start(out=WV_sb[:], in_=wplane(wv_d))
        nc.sync.dma_start(out=MB_sb[:, 2:4], in_=mb_r[:, 2:4])
        nc.sync.dma_start(out=XTQ_sb[:, :, QC:QC + 512],
                          in_=xtq_r[:, :, QC:QC + 512])
        nc.sync.dma_start(out=MB_sb[:, 4:6], in_=mb_r[:, 4:6])
        nc.sync.dma_start(out=XTQ_sb[:, :, QC + 512:S],
                          in_=xtq_r[:, :, QC + 512:S])
        nc.sync.dma_start(out=MB_sb[:, 6:8], in_=mb_r[:, 6:8])
        nc.sync.dma_start(out=XQ_sb[:], in_=xq_d[0:512].rearrange(
            "(q p) d -> p q d", p=128))
        nc.sync.dma_start(out=WH_sb[:], in_=wplane(wfh_d))
        nc.sync.dma_start(out=WL_sb[:], in_=wplane(wfl_d))

        # V pad memsets (Pool, runs from t=0; only the 63 pad cols + ones)
        for sp in range(SP):
            vre = V_dr[sp].rearrange("p i (h c) -> p i h c", c=VB)
            nc.gpsimd.memset(vre[:, :, :, DH + 1:], 0.0)
            nc.gpsimd.memset(vre[:, :, :, DH:DH + 1], 1.0)

        # ---------- projection work units (each atomic in pPS1) ----------
        def k_unit(dt, kc, act):
            """KT[dt][:, kc*512:+512] from a 3-matmul DR chain."""
            ps = pPS1.tile([128, 512], F32, name="ps1")
            for c in range(C2):
                nc.tensor.matmul(
                    ps[:],
                    lhsT=WK_sb[:, 2 * c:2 * c + 2, dt * 128:(dt + 1) * 128],
                    rhs=XTQ_sb[:, 2 * c:2 * c + 2, kc * 512:(kc + 1) * 512],
                    start=(c == 0), stop=(c == C2 - 1), perf_mode=DRM)
            dst = KT_sb[dt][:, kc * 512:(kc + 1) * 512]
            if act:
                nc.scalar.copy(dst, ps[:])
            else:
                nc.vector.tensor_copy(dst, ps[:])

        def q_unit(dt, qc, act):
            ps = pPS1.tile([128, 512], F32, name="ps1")
            for c in range(C2):
                nc.tensor.matmul(
                    ps[:],
                    lhsT=WQ_sb[:, 2 * c:2 * c + 2, dt * 128:(dt + 1) * 128],
                    rhs=XTQ_sb[:, 2 * c:2 * c + 2, qc * 512:(qc + 1) * 512],
                    start=(c == 0), stop=(c == C2 - 1), perf_mode=DRM)
            dst = QT_sb[dt][:, qc * 512:(qc + 1) * 512]
            if act:
                nc.scalar.copy(dst, ps[:])
            else:
                nc.vector.tensor_copy(dst, ps[:])

        def v_unit(st, act):
            """V rows for key tile st (both 384-col halves)."""
            for j in range(2):
                ps = pPS1.tile([128, 512], F32, name="ps1")[:, 0:384]
                for c in range(C2):
                    nc.tensor.matmul(
                        ps[:],
                        lhsT=XTQ_sb[:, 2 * c:2 * c + 2, st * 128:(st + 1) * 128],
                        rhs=WV_sb[:, 2 * c:2 * c + 2, j * 384:(j + 1) * 384],
                        start=(c == 0), stop=(c == C2 - 1), perf_mode=DRM)
                dst = V_dr[st // 2].rearrange(
                    "p i (h c) -> p i h c", c=VB)[
                    :, st % 2, j * 6:(j + 1) * 6, 0:DH]
                src = ps.rearrange("p (h c) -> p h c", c=DH)
                if act:
                    nc.scalar.copy(dst, src)
                else:
                    nc.vector.tensor_copy(dst, src)

        # ---------- attention ----------
        cxr = {}

        def attention(h, qc, pump, at2, at4):
            """One head: 16 mask/score matmuls + 8 exp. The ctx matmuls and
            the transpose/normalize tail are closures emitted during the
            NEXT head (at2/at4 hooks) so V-eviction lag and serial tails
            cannot stall the in-order PE queue ahead of these scores."""
            dtile, doff = h // 2, (h % 2) * DH
            qsl = slice(qc * 512, (qc + 1) * 512)
            pts = []
            for t in range(SP):
                ps = pPS.tile([128, 2, 512], F32, name="ps")
                for j in range(2):
                    kt = 2 * t + j
                    nc.tensor.matmul(
                        ps[:, j, :],
                        lhsT=i2[:, j, :, :],
                        rhs=MB_dr[t][:, :, qsl],
                        start=True, stop=False, perf_mode=DRM)
                    nc.tensor.matmul(
                        ps[:, j, :],
                        lhsT=KT_sb[dtile][doff:doff + DH,
                                          kt * 128:(kt + 1) * 128],
                        rhs=QT_sb[dtile][doff:doff + DH, qsl],
                        start=False, stop=True)
                pt = pPT.tile([128, 2, 512], FP8, name="pt")
                nc.scalar.activation(pt[:], ps[:], AF.Exp, scale=1.0 / 8.0)
                pts.append(pt)
                if t == 2 and at2 is not None:
                    at2()
                if t == 4 and at4 is not None:
                    at4()
                pump(t)

            def ctxf(h=h, pts=pts):
                pc = pPC.tile([128, 512], F32, name="pc")
                for t in range(SP):
                    nc.tensor.matmul(
                        pc[:],
                        lhsT=V_dr[t][:, :, h * VB:(h + 1) * VB],
                        rhs=pts[t][:],
                        start=(t == 0), stop=(t == SP - 1), perf_mode=DRM)
                # ctx^T: rows 0..63 ctx, row 64 softmax denominators
                cx = pCX.tile([128, 512], BF16, name="cx")
                nc.vector.tensor_copy(cx[:], pc[:])
                cxr[h] = cx

            def tail(h=h, qc=qc):
                cx = cxr.pop(h)
                tp = pPS1.tile([128, 512], BF16, name="ps1")
                for qtl in range(4):
                    nc.tensor.transpose(
                        tp[:, qtl * 128:(qtl + 1) * 128],
                        cx[:, qtl * 128:(qtl + 1) * 128], ident[:])
                rs = pRS.tile([128, 4], F32, name="rs")
                for qtl in range(4):
                    qt = qc * 4 + qtl
                    nc.vector.reciprocal(rs[:, qtl:qtl + 1],
                                         tp[:, qtl * 128 + DH:qtl * 128 + DH + 1])
                    nc.vector.tensor_scalar_mul(
                        AT_sb[qt][:, h * DH:(h + 1) * DH],
                        tp[:, qtl * 128:qtl * 128 + DH], rs[:, qtl:qtl + 1])
            return ctxf, tail

        # ---------- LN1 + FFN + LN2-input chunks per query tile ----------
        def c1a_unit(qt):
            """u = AT+XQ; U2[qt] = bf16(u-mean) [doubles as the FFN
            residual]. LN1 needs no variance: the 1/std scale cancels
            through the linear FFN into scale-invariant LN2."""
            sm = pST.tile([128, 2], F32, name="sm")
            nc.vector.tensor_tensor(AT_sb[qt][:], AT_sb[qt][:],
                                    XQ_sb[:, qt % 4], ALU.add)
            nc.vector.tensor_reduce(sm[:, 0:1], AT_sb[qt][:],
                                    mybir.AxisListType.X, ALU.add)
            nc.gpsimd.tensor_scalar_mul(sm[:, 1:2], sm[:, 0:1], 1.0 / D)
            nc.vector.tensor_scalar(U2_sb[qt][:], AT_sb[qt][:], sm[:, 1:2],
                                    None, ALU.subtract)

        def h8_unit(qt, pool_lb):
            h8 = pSC.tile([128, D], FP8, name="h8")
            nc.gpsimd.tensor_copy(h8[:], U2_sb[qt][:])
            lb = pSC.tile([128, D], BF16, name="lb")
            if pool_lb:
                nc.gpsimd.tensor_tensor(lb[:], U2_sb[qt][:], h8[:],
                                        ALU.subtract)
            else:
                nc.vector.tensor_tensor(lb[:], U2_sb[qt][:], h8[:],
                                        ALU.subtract)
            return lb

        def c1_unit(qt):
            c1a_unit(qt)
            return h8_unit(qt, pool_lb=True)

        def c2_one(qt, src_t, dr, cp, act):
            """Transpose d-chunk pair (2cp, 2cp+1) of src into both planes
            of dr[cp]: 2 transposes into one PSUM tile, 1 paired evict."""
            tp = pPS1.tile([128, 512], BF16, name="ps1")
            for k in range(2):
                c = 2 * cp + k
                nc.tensor.transpose(
                    tp[:, k * 128:(k + 1) * 128],
                    src_t[:, c * 128:(c + 1) * 128], ident[:])
            psrc = tp[:, 0:256].rearrange("p (i f) -> p i f", i=2)
            pdst = dr[cp][:, :, qt * 128:(qt + 1) * 128]
            if act:
                nc.scalar.copy(pdst, psrc)
            else:
                nc.vector.tensor_copy(pdst, psrc)

        def c2_unit(qt, lb, cp, act):
            c2_one(qt, U2_sb[qt], FTH_dr, cp, act)
            c2_one(qt, lb, FTL_dr, cp, act)

        def c3_units(qt, act=False):
            """FFN j-halves as 2 atomic PE units + a stats closure."""
            r2 = pR2.tile([128, D], BF16, name="r2")

            def mmj(j, qt=qt):
                def go():
                    ps = pPS1.tile([128, 512], F32, name="ps1")[:, 0:384]
                    kk = 0
                    for (A, W) in ((FTH_dr, WH_dr), (FTH_dr, WL_dr),
                                   (FTL_dr, WH_dr)):
                        for c in range(C2):
                            nc.tensor.matmul(
                                ps[:],
                                lhsT=A[c][:, :, qt * 128:(qt + 1) * 128],
                                rhs=W[c][:, :, j * 384:(j + 1) * 384],
                                start=(kk == 0), stop=(kk == 3 * C2 - 1),
                                perf_mode=DRM)
                            kk += 1
                    if act:
                        nc.scalar.activation(
                            r2[:, j * 384:(j + 1) * 384], ps[:], AF.Copy,
                            scale=1.0 / 32.0)
                    else:
                        nc.vector.tensor_scalar_mul(
                            r2[:, j * 384:(j + 1) * 384], ps[:], 1.0 / 32.0)
                return go

            def fin(qt=qt):
                nc.vector.tensor_tensor(U2_sb[qt][:], U2_sb[qt][:], r2[:],
                                        ALU.add)
                st6 = pST.tile([128, 2, 6], F32, name="st6")
                nc.vector.bn_stats(st6[:, 0, :], U2_sb[qt][:, 0:D // 2])
                nc.vector.bn_stats(st6[:, 1, :], U2_sb[qt][:, D // 2:])
                nc.vector.bn_aggr(MV_sb[:, qt], st6[:])
            return [mmj(0), mmj(1), fin]

        def ln2_apply(qt):
            """(u2 - mean) * rstd -> OT (bf16). qt0-3 run on ACT right
            after the last exp (fills its bubble); qt4-7 on DVE so the
            last chains don't serialize behind ACT's eviction queue."""
            if qt < 4:
                nm = pRS.tile([128, 1], F32, name="nm")
                nc.vector.tensor_scalar(nm[:], MV_sb[:, qt, 0:1],
                                        RS_sb[:, qt:qt + 1], -1.0,
                                        ALU.mult, ALU.mult)
                nc.scalar.activation(OT_sb[:, qt % 4], U2_sb[qt][:],
                                     AF.Identity,
                                     scale=RS_sb[:, qt:qt + 1], bias=nm[:])
            else:
                nc.vector.tensor_scalar(OT_sb[:, qt % 4], U2_sb[qt][:],
                                        MV_sb[:, qt, 0:1],
                                        RS_sb[:, qt:qt + 1],
                                        ALU.subtract, ALU.mult)

        # ---------- emission schedule ----------
        # Prologue: K dt0 cols 0:1024 + Q dt0 qsl0 (ACT evicts; ACT idle).
        k_unit(0, 0, act=True)
        q_unit(0, 0, act=True)
        k_unit(0, 1, act=False)

        # h0 fillers: V st0-13 at 2/t, K dt0 kc2-3 squeezed on t2-3;
        # V st14-15 land on h1 t0-1, just before ctx(h0) fires at h1 t2.
        h0_sched = {
            0: [lambda: v_unit(0, False), lambda: v_unit(1, False)],
            1: [lambda: v_unit(2, False), lambda: v_unit(3, False)],
            2: [lambda: v_unit(4, False), lambda: k_unit(0, 2, False)],
            3: [lambda: v_unit(5, False), lambda: k_unit(0, 3, False)],
            4: [lambda: v_unit(6, False), lambda: v_unit(7, False)],
            5: [lambda: v_unit(8, False), lambda: v_unit(9, False)],
            6: [lambda: v_unit(10, False), lambda: v_unit(11, False)],
            7: [lambda: v_unit(12, False), lambda: v_unit(13, False)],
        }
        h1_sched = {
            0: [lambda: v_unit(14, False)],
            1: [lambda: v_unit(15, False)],
        }

        fillers = deque()
        fillers.append(lambda: q_unit(0, 1, False))
        for dt in range(1, CD):
            fillers.append(lambda dt=dt: q_unit(dt, 0, False))
            fillers.append(lambda dt=dt: q_unit(dt, 1, False))
            for kc in range(4):
                fillers.append(lambda dt=dt, kc=kc: k_unit(dt, kc, False))

        def pump(t):
            if fillers:
                fillers.popleft()()

        def pump_slow(t):
            if t % 2 and fillers:
                fillers.popleft()()

        def pump_h0(t):
            for u in h0_sched.get(t, ()):
                u()

        def pump_h1(t):
            for u in h1_sched.get(t, ()):
                u()
            pump(t)

        carry = {}

        def queue_halftail(qh):
            for qtl in range(4):
                qt = qh * 4 + qtl

                def c1c(qt=qt):
                    carry[qt] = c1_unit(qt)

                def c3all(qt=qt):
                    carry.pop(qt)
                    for u in reversed(c3_units(qt)):
                        fillers.appendleft(u)

                fillers.append(c1c)
                for cp in range(3):
                    fillers.append(
                        lambda qt=qt, cp=cp: c2_unit(qt, carry[qt], cp,
                                                     act=False))
                fillers.append(c3all)

        p_ctx, p_tail = None, None
        for qc in (0, 1):
            for h in range(H):
                if qc == 0 and h == 0:
                    pmp = pump_h0
                elif qc == 0 and h == 1:
                    pmp = pump_h1
                elif qc == 0:
                    pmp = pump_slow
                else:
                    pmp = pump
                p_ctx, p_tail = attention(h, qc, pmp, p_ctx, p_tail)
                if qc == 1 and h == 0:
                    queue_halftail(0)
                if qc == 1 and h == 6:
                    # refill the residual buffer with the second query half
                    nc.sync.dma_start(
                        out=XQ_sb[:], in_=xq_d[512:1024].rearrange(
                            "(q p) d -> p q d", p=128))

        # last ctx + tail; then finish qt0-3 (their stats are done) while
        # the qt4-7 chains run, all sqrts batched in one ACT table context.
        p_ctx()
        p_tail()
        while fillers:
            fillers.popleft()()
        sd03 = pRS.tile([128, 4], F32, name="sd03")
        nc.scalar.activation(sd03[:], MV_sb[:, 0:4, 1], AF.Sqrt, bias=eps_t[:])
        nc.vector.reciprocal(RS_sb[:, 0:4], sd03[:])
        for qt in range(4):
            ln2_apply(qt)
        nc.sync.dma_start(
            out=out_d[0:512, :].rearrange("(q p) d -> p q d", p=128),
            in_=OT_sb[:])
        # qt4-7 chains, stage round-robin: c1a -> hb-transposes (run
        # while h8/lb compute) -> lb-transposes -> FFN -> stats -> LN2
        QF = (4, 5, 6, 7)
        for qt in QF:
            c1a_unit(qt)
        lbs = {}
        lbs[4] = h8_unit(4, pool_lb=True)
        for cp in range(3):
            c2_one(4, U2_sb[4], FTH_dr, cp, act=True)
        lbs[5] = h8_unit(5, pool_lb=False)
        for cp in range(3):
            c2_one(5, U2_sb[5], FTH_dr, cp, act=True)
        lbs[6] = h8_unit(6, pool_lb=True)
        for cp in range(3):
            c2_one(4, lbs[4], FTL_dr, cp, act=True)
        lbs[7] = h8_unit(7, pool_lb=False)
        for cp in range(3):
            c2_one(6, U2_sb[6], FTH_dr, cp, act=True)
        c3u = {qt: c3_units(qt, act=True) for qt in QF}
        c3u[4][0]()
        for cp in range(3):
            c2_one(5, lbs[5], FTL_dr, cp, act=True)
        c3u[4][1]()
        for cp in range(3):
            c2_one(7, U2_sb[7], FTH_dr, cp, act=True)
        c3u[5][0]()
        for cp in range(3):
            c2_one(6, lbs[6], FTL_dr, cp, act=True)
        c3u[5][1]()
        c3u[4][2]()
        for cp in range(3):
            c2_one(7, lbs[7], FTL_dr, cp, act=True)
        c3u[6][0]()
        c3u[6][1]()
        c3u[5][2]()

        def fin2(qt):
            sd = pRS.tile([128, 1], F32, name="sd")
            nc.scalar.activation(sd[:], MV_sb[:, qt, 1:2], AF.Sqrt,
                                 bias=eps_t[:])
            nc.vector.reciprocal(RS_sb[:, qt:qt + 1], sd[:])
            ln2_apply(qt)

        fin2(4)
        c3u[7][0]()
        c3u[7][1]()
        c3u[6][2]()
        fin2(5)
        c3u[7][2]()
        fin2(6)
        fin2(7)
        nc.sync.dma_start(
            out=out_d[512:1024, :].rearrange("(q p) d -> p q d", p=128),
            in_=OT_sb[:])

    return nc


_built = {}


def _get_nc():
    if "nc" not in _built:
        _built["nc"] = build_nc()
    return _built["nc"]


def _make_in_maps(inputs):
    f8 = ml_dtypes.float8_e4m3
    x = np.asarray(inputs["inputs"], np.float32)
    mask = np.asarray(inputs["attn_mask"], bool)
    wq = np.asarray(inputs["Wq"], np.float32).astype(f8)
    wk = np.asarray(inputs["Wk"], np.float32).astype(f8)
    wv = np.asarray(inputs["Wv"], np.float32).astype(f8)
    # fused linear FFN: Ws = 32*(W1@W2), hi/lo fp8 split for compensation
    ws = 32.0 * (np.asarray(inputs["W1"], np.float32)
                 @ np.asarray(inputs["W2"], np.float32))
    wh = ws.astype(f8)
    wl = (ws - wh.astype(np.float32)).astype(f8)
    wfh, wfl = wh, wl

    i2 = np.zeros((128, 2, 2, 128), np.float32)
    for p in range(128):
        for i in range(2):
            i2[p, p // 64, i, i * 64 + p % 64] = MNEG
    i2 = np.ascontiguousarray(i2.reshape(128, 512)).astype(f8)

    # The device program folds no affine terms: this module's initialization
    # has them all zero / one -- assert the cases we don't emit.
    for name in ("bq", "bk", "bv", "b1", "b2", "beta1", "beta2"):
        assert not np.asarray(inputs[name]).any(), f"{name} nonzero unsupported"
    for name in ("g1", "g2"):
        assert np.allclose(np.asarray(inputs[name]), 1.0), f"{name} != 1 unsupported"

    in_maps = []
    for core in range(NCORES):
        b, qh = core // 2, core % 2
        q0 = qh * QC
        # Per-core sequence permutation: own queries first (cols 0..1023) so
        # the SPMD program can slice Q at a fixed offset. K/V/mask all use
        # the same permuted key order; softmax is key-order invariant.
        perm = np.r_[np.arange(q0, q0 + QC),
                     np.arange(0, q0),
                     np.arange(q0 + QC, S)]
        xtq8 = np.ascontiguousarray(x[b].T[:, perm]).astype(f8)
        xq = np.ascontiguousarray(x[b, q0:q0 + QC]).astype(ml_dtypes.bfloat16)
        # mask8[t*128 + j*64 + r, i*1024 + q] = mkq[(2t+j)*128 + i*64+r, q]
        mkq = mask[b][q0:q0 + QC][:, perm].T.astype(np.float32)  # [k(perm), q]
        m5 = mkq.reshape(8, 2, 2, 64, QC).transpose(0, 1, 3, 2, 4)
        mask8 = np.ascontiguousarray(m5.reshape(1024, 2 * QC)).astype(f8)
        in_maps.append(dict(xtq8=xtq8, xq=xq, mask8=mask8, i2=i2,
                            wq8=wq, wk8=wk, wv8=wv, wfh8=wfh, wfl8=wfl))
    return in_maps


def _run(in_maps):
    from concourse.bass_utils import run_bass_kernel_spmd
    nc = _get_nc()
    return run_bass_kernel_spmd(nc, in_maps, list(range(NCORES)))


def _assemble(res):
    out = np.empty((B, S, D), np.float32)
    for core in range(NCORES):
        b, qh = core // 2, core % 2
        out[b, qh * QC:(qh + 1) * QC] = res.results[core]["out"].astype(
            np.float32)
    return out


def kernel(**inputs) -> np.ndarray:
    return _assemble(_run(_make_in_maps(inputs)))
